# revision 13
# baseline (speedup 1.0000x reference)
"""Distributed multi-head attention kernel for 8 TRN2 NeuronCores.

Reference problem (hardcoded):
    hidden_states [1, 1024, 1, 2048] f32, Wq/Wk/Wv [1024, 1024],
    Wo [1024, 1024], bo [1024].  16 heads x 64 dim, seq 2048.

Sharding: tensor-parallel over heads.  Core i computes heads (2i, 2i+1):
  - QKV projections for its 128 channels (scale 1/8 folded into Wq),
  - scores transposed S_T[k, q] per head (no max subtraction; scores ~ N(0,1)),
  - one exp per key chunk on ScalarE ([128, 1024] double-buffered PSUM),
  - PV with a ones-column appended to vT so the softmax denominator falls out
    of the same PSUM accumulation,
  - normalize (reciprocal + partition-broadcast + multiply),
  - AllGather of the 1 MB attn block,
  - row shard of the output projection (+bias) -> out rows 128i..128(i+1).
Host concatenates the 8 row shards.

build(loop_r=N) wraps the pre-collective body and the post-collective
projection in hardware For_i loops (N iterations each) for wall-clock
benchmarking; the collective itself runs once (not allowed in control flow).
"""

import contextlib

import numpy as np

import concourse.bass as bass
import concourse.mybir as mybir
import concourse.tile as tile
from concourse import bacc
from concourse.bass import ts, ds
from concourse.bass_utils import run_bass_kernel_spmd

# Problem constants (hardcoded per harness contract)
S = 2048          # sequence length
C = 1024          # query dim == inner dim
P = 128           # partitions / per-core channel count
D = 64            # head dim
HC = 2            # heads per core
N_CORES = 8
KC = C // P       # 8 contraction chunks for the projections
NKT = S // P      # 16 key-position chunks
NB = S // 512     # 4 free-dim blocks of 512
FP32 = mybir.dt.float32
FPR = mybir.dt.float32r
AFT = mybir.ActivationFunctionType


def build(loop_r=None):
    nc = bacc.Bacc("TRN2", target_bir_lowering=False, debug=False,
                   num_devices=N_CORES)
    x_d = nc.dram_tensor("x", [C, S], FPR, kind="ExternalInput")
    wq_d = nc.dram_tensor("wqT", [C, P], FPR, kind="ExternalInput")
    wk_d = nc.dram_tensor("wkT", [C, P], FPR, kind="ExternalInput")
    wv_d = nc.dram_tensor("wvT", [C, P], FPR, kind="ExternalInput")
    wo_d = nc.dram_tensor("woT", [C, P], FPR, kind="ExternalInput")
    bo_d = nc.dram_tensor("bo", [P, 1], FP32, kind="ExternalInput")
    id_d = nc.dram_tensor("ident", [P, P], FPR, kind="ExternalInput")
    ones_d = nc.dram_tensor("ones", [P, 1], FPR, kind="ExternalInput")
    out_d = nc.dram_tensor("out", [P, S], FP32, kind="ExternalOutput")

    with tile.TileContext(nc) as tc:
        with (
            tc.tile_pool(name="const", bufs=1) as cpool,
            tc.tile_pool(name="big", bufs=1) as big,
            tc.tile_pool(name="opsum", bufs=4, space="PSUM") as opool,
            tc.tile_pool(name="stpsum", bufs=2, space="PSUM") as stpool,
            tc.tile_pool(name="exp", bufs=3) as epool,
            tc.tile_pool(name="small", bufs=4) as spool,
            tc.tile_pool(name="rhs", bufs=4) as rpool,
            tc.tile_pool(name="dram", bufs=1, space="DRAM") as dpool,
        ):
            # ---- constants / weights (outside any bench loop) ----
            ident = cpool.tile([P, P], FPR)
            nc.sync.dma_start(ident[:], id_d.ap())
            ones_sb = cpool.tile([P, 1], FPR, tag="ones")
            nc.sync.dma_start(ones_sb[:], ones_d.ap())
            w_sb = {}
            for name, dram in (("q", wq_d), ("k", wk_d), ("v", wv_d),
                               ("o", wo_d)):
                t = cpool.tile([P, KC, P], FPR, tag=f"w{name}")
                nc.sync.dma_start(
                    t[:], dram.ap().rearrange("(kc p) m -> p kc m", p=P))
                w_sb[name] = t
            bo_sb = cpool.tile([P, 1], FP32, tag="bo")
            nc.sync.dma_start(bo_sb[:], bo_d.ap())

            x_sb = big.tile([P, KC, S], FPR, tag="x")
            proj = {}
            for name in ("k", "q", "v"):
                proj[name] = big.tile([P, S], FPR, tag=f"{name}sb",
                                      name=f"{name}sb")
            vTa = big.tile([P, HC, NKT, D + 1], FPR, tag="vTa")
            attn_sb = big.tile([P, S], FPR, tag="attn")
            out_sb = big.tile([P, S], FP32, tag="outsb")
            ag_in = dpool.tile([P, S], FPR, tag="agin")
            ag_out = dpool.tile([C, S], FPR, tag="agout",
                                addr_space="Shared")

            def emit_pre():
                # x into SBUF in (kc, nb) sub-chunks so QKV chases the DMAs
                x_view = x_d.ap().rearrange("(kc p) s -> kc p s", kc=KC)
                for nb in range(NB):
                    for kc in range(KC):
                        nc.sync.dma_start(x_sb[:, kc, ts(nb, 512)],
                                          x_view[kc][:, ts(nb, 512)])
                # QKV projections, nb-major
                for nb in range(NB):
                    for name in ("k", "q", "v"):
                        ps = opool.tile([P, 512], FP32, tag="o", name="qkv_ps")
                        for kc in range(KC):
                            nc.tensor.matmul(
                                ps[:], w_sb[name][:, kc, :],
                                x_sb[:, kc, ts(nb, 512)],
                                start=(kc == 0), stop=(kc == KC - 1))
                        nc.vector.tensor_copy(proj[name][:, ts(nb, 512)],
                                              ps[:])
                q_sb, k_sb, v_sb = proj["q"], proj["k"], proj["v"]

                # vT with ones column
                nc.vector.tensor_copy(
                    vTa[:, :, :, D:D + 1],
                    ones_sb[:, None, None, :].broadcast_to([P, HC, NKT, 1]))
                for kt in range(NKT):
                    tp = opool.tile([P, P], FPR, tag="o", name="tp")
                    nc.tensor.transpose(tp[:], v_sb[:, ts(kt, P)], ident[:])
                    for h in range(HC):
                        nc.vector.tensor_copy(vTa[:, h, kt, 0:D],
                                              tp[:, h * D:(h + 1) * D])

                # attention per (head, q-half)
                for h in range(HC):
                    hsl = slice(h * D, (h + 1) * D)
                    for qh in range(2):
                        o_ps = [opool.tile([P, 512], FP32, tag="o",
                                           name=f"o_ps_h{h}_{qh}_{qb}")
                                for qb in range(2)]
                        for kt in range(NKT):
                            st = stpool.tile([P, 1024], FP32, tag="st")
                            for nb in range(2):
                                nc.tensor.matmul(
                                    st[:, ts(nb, 512)],
                                    k_sb[hsl, ts(kt, P)],
                                    q_sb[hsl, ds(qh * 1024 + nb * 512,
                                                   512)],
                                    start=True, stop=True)
                            e = epool.tile([P, 1024], FPR, tag="e")
                            nc.scalar.activation(e[:], st[:], AFT.Exp)
                            for qb in range(2):
                                nc.tensor.matmul(
                                    o_ps[qb][0:D + 1, :],
                                    vTa[:, h, kt, :],
                                    e[:, ts(qb, 512)],
                                    start=(kt == 0), stop=(kt == NKT - 1))
                        # normalize: rows 0:64 attn, row 64 denominator
                        for qb in range(2):
                            qsl = ds(qh * 1024 + qb * 512, 512)
                            rec = spool.tile([1, 512], FP32, tag="rec",
                                             name="rec")
                            nc.vector.reciprocal(rec[:],
                                                 o_ps[qb][D:D + 1, :])
                            bc = spool.tile([D, 512], FP32, tag="bc",
                                            name="bc")
                            nc.gpsimd.partition_broadcast(bc[:], rec[:])
                            nc.vector.tensor_mul(attn_sb[hsl, qsl],
                                                 o_ps[qb][0:D, :], bc[:])
                nc.sync.dma_start(ag_in[:], attn_sb[:])

            def emit_post():
                for nb in range(NB):
                    o_ps = opool.tile([P, 512], FP32, tag="o", name="out_ps")
                    for kc in range(KC):
                        rt = rpool.tile([P, 512], FPR, tag="rhs", name="rt")
                        nc.sync.dma_start(
                            rt[:], ag_out[ds(kc * P, P), ts(nb, 512)])
                        nc.tensor.matmul(o_ps[:], w_sb["o"][:, kc, :],
                                         rt[:],
                                         start=(kc == 0), stop=(kc == KC - 1))
                    nc.vector.tensor_scalar_add(out_sb[:, ts(nb, 512)],
                                                o_ps[:], bo_sb[:])
                    nc.sync.dma_start(out_d.ap()[:, ts(nb, 512)],
                                      out_sb[:, ts(nb, 512)])

            if loop_r is None:
                emit_pre()
            else:
                with tc.For_i(0, loop_r, 1):
                    emit_pre()
            nc.gpsimd.collective_compute(
                "AllGather", mybir.AluOpType.bypass,
                ins=[ag_in.opt()], outs=[ag_out.opt()],
                replica_groups=[list(range(N_CORES))])
            if loop_r is None:
                emit_post()
            else:
                with tc.For_i(0, loop_r, 1):
                    emit_post()
    nc.finalize()
    return nc


_NC = None


def _get_nc():
    global _NC
    if _NC is None:
        _NC = build()
    return _NC


def make_in_maps(hidden_states, Wq, Wk, Wv, Wo, bo):
    x = np.ascontiguousarray(
        np.asarray(hidden_states, np.float32).reshape(C, S))
    scale = np.float32(D ** -0.5)
    Wq = np.asarray(Wq, np.float32)
    Wk = np.asarray(Wk, np.float32)
    Wv = np.asarray(Wv, np.float32)
    Wo = np.asarray(Wo, np.float32)
    bo = np.asarray(bo, np.float32)
    in_maps = []
    for i in range(N_CORES):
        sl = slice(i * P, (i + 1) * P)
        in_maps.append({
            "x": x,
            "wqT": np.ascontiguousarray((Wq[sl] * scale).T),
            "wkT": np.ascontiguousarray(Wk[sl].T),
            "wvT": np.ascontiguousarray(Wv[sl].T),
            "woT": np.ascontiguousarray(Wo[sl].T),
            "bo": np.ascontiguousarray(bo[sl].reshape(P, 1)),
            "ident": np.eye(P, dtype=np.float32),
            "ones": np.ones((P, 1), np.float32),
        })
    return in_maps


def kernel(hidden_states, Wq, Wk, Wv, Wo, bo):
    nc = _get_nc()
    in_maps = make_in_maps(hidden_states, Wq, Wk, Wv, Wo, bo)
    res = run_bass_kernel_spmd(nc, in_maps, core_ids=list(range(N_CORES)))
    out = np.concatenate([res.results[i]["out"] for i in range(N_CORES)],
                         axis=0)
    return out.reshape(1, C, 1, S)


# revision 17
# speedup vs baseline: 4.2034x; 4.2034x over previous
"""Distributed multi-head attention kernel for 8 TRN2 NeuronCores.

Reference problem (hardcoded):
    hidden_states [1, 1024, 1, 2048] f32, Wq/Wk/Wv [1024, 1024],
    Wo [1024, 1024], bo [1024].  16 heads x 64 dim, seq 2048.

Sharding: tensor-parallel over heads.  Core i computes heads (2i, 2i+1):
  - QKV projections for its 128 channels (scale 1/8 folded into Wq),
  - scores transposed S_T[k, q] per head (no max subtraction; scores ~ N(0,1)),
  - one exp per key chunk on ScalarE ([128, 1024] double-buffered PSUM),
  - PV with a ones-column appended to vT so the softmax denominator falls out
    of the same PSUM accumulation,
  - normalize (reciprocal + partition-broadcast + multiply),
  - AllGather of the 1 MB attn block,
  - row shard of the output projection (+bias) -> out rows 128i..128(i+1).
Host concatenates the 8 row shards.

build(loop_r=N) wraps the pre-collective body and the post-collective
projection in hardware For_i loops (N iterations each) for wall-clock
benchmarking; the collective itself runs once (not allowed in control flow).
"""

import contextlib

import numpy as np
import ml_dtypes

import concourse.bass as bass
import concourse.mybir as mybir
import concourse.tile as tile
from concourse import bacc
from concourse.bass import ts, ds
from concourse.bass_utils import run_bass_kernel_spmd

# Problem constants (hardcoded per harness contract)
S = 2048          # sequence length
C = 1024          # query dim == inner dim
P = 128           # partitions / per-core channel count
D = 64            # head dim
HC = 2            # heads per core
N_CORES = 8
KC = C // P       # 8 contraction chunks for the projections
NKT = S // P      # 16 key-position chunks
NB = S // 512     # 4 free-dim blocks of 512
FP32 = mybir.dt.float32
FPR = mybir.dt.float32r
BF16 = mybir.dt.bfloat16
AFT = mybir.ActivationFunctionType


def build(loop_r=None, part="full"):
    nc = bacc.Bacc("TRN2", target_bir_lowering=False, debug=False,
                   num_devices=N_CORES)
    x_d = nc.dram_tensor("x", [C, S], BF16, kind="ExternalInput")
    wq_d = nc.dram_tensor("wqT", [C, P], BF16, kind="ExternalInput")
    wk_d = nc.dram_tensor("wkT", [C, P], BF16, kind="ExternalInput")
    wv_d = nc.dram_tensor("wvT", [C, P], BF16, kind="ExternalInput")
    wo_d = nc.dram_tensor("woT", [C, P], BF16, kind="ExternalInput")
    bo_d = nc.dram_tensor("bo", [P, 1], FP32, kind="ExternalInput")
    id_d = nc.dram_tensor("ident", [P, P], FPR, kind="ExternalInput")
    ones_d = nc.dram_tensor("ones", [P, 1], FPR, kind="ExternalInput")
    out_d = nc.dram_tensor("out", [P, S], FP32, kind="ExternalOutput")

    with tile.TileContext(nc) as tc:
        with (
            tc.tile_pool(name="const", bufs=1) as cpool,
            tc.tile_pool(name="big", bufs=1) as big,
            tc.tile_pool(name="opsum", bufs=4, space="PSUM") as opool,
            tc.tile_pool(name="stpsum", bufs=2, space="PSUM") as stpool,
            tc.tile_pool(name="exp", bufs=3) as epool,
            tc.tile_pool(name="small", bufs=4) as spool,
            tc.tile_pool(name="rhs", bufs=4) as rpool,
            tc.tile_pool(name="dram", bufs=1, space="DRAM") as dpool,
        ):
            # ---- constants / weights (outside any bench loop) ----
            ident = cpool.tile([P, P], FPR)
            nc.sync.dma_start(ident[:], id_d.ap())
            ones_sb = cpool.tile([P, 1], FPR, tag="ones")
            nc.sync.dma_start(ones_sb[:], ones_d.ap())
            w_sb = {}
            for name, dram in (("q", wq_d), ("k", wk_d), ("v", wv_d),
                               ("o", wo_d)):
                t = cpool.tile([P, KC, P], BF16, tag=f"w{name}")
                nc.sync.dma_start(
                    t[:], dram.ap().rearrange("(kc p) m -> p kc m", p=P))
                w_sb[name] = t
            bo_sb = cpool.tile([P, 1], FP32, tag="bo")
            nc.sync.dma_start(bo_sb[:], bo_d.ap())
            # absorb the exp table load into the DMA lead-in
            warm = cpool.tile([P, 1], FP32, tag="warm")
            nc.scalar.activation(warm[:], bo_sb[:], AFT.Exp)

            x_sb = big.tile([P, KC, S], BF16, tag="x")
            proj = {}
            for name in ("k", "q", "v"):
                proj[name] = big.tile([P, S], FPR, tag=f"{name}sb",
                                      name=f"{name}sb")
            vTa = big.tile([P, HC, NKT, D + 1], FPR, tag="vTa")
            attn_sb = big.tile([P, S], BF16, tag="attn")
            out_sb = big.tile([P, S], FP32, tag="outsb")
            ag_in = [dpool.tile([P, S // 2], BF16, tag=f"agin{i}",
                                name=f"agin{i}") for i in range(2)]
            ag_out = [dpool.tile([C, S // 2], BF16, tag=f"agout{i}",
                                 addr_space="Shared", name=f"agout{i}")
                      for i in range(2)]

            def emit_xdma():
                # x into SBUF in (kc, nb) sub-chunks so QKV chases the DMAs
                x_view = x_d.ap().rearrange("(kc p) s -> kc p s", kc=KC)
                for nb in range(NB):
                    for kc in range(KC):
                        nc.sync.dma_start(x_sb[:, kc, ts(nb, 512)],
                                          x_view[kc][:, ts(nb, 512)])

            def emit_qkv():
                # QKV projections, nb-major
                for nb in range(NB):
                    for name in ("k", "q", "v"):
                        ps = opool.tile([P, 512], FP32, tag="o", name="qkv_ps")
                        for kc in range(KC):
                            nc.tensor.matmul(
                                ps[:], w_sb[name][:, kc, :],
                                x_sb[:, kc, ts(nb, 512)],
                                start=(kc == 0), stop=(kc == KC - 1))
                        nc.vector.tensor_copy(proj[name][:, ts(nb, 512)],
                                              ps[:])
            def emit_vta():
                v_sb = proj["v"]
                # vT with ones column
                nc.vector.tensor_copy(
                    vTa[:, :, :, D:D + 1],
                    ones_sb[:, None, None, :].broadcast_to([P, HC, NKT, 1]))
                for kt in range(NKT):
                    tp = opool.tile([P, P], FPR, tag="o", name="tp")
                    nc.tensor.transpose(tp[:], v_sb[:, ts(kt, P)], ident[:])
                    for h in range(HC):
                        nc.vector.tensor_copy(vTa[:, h, kt, 0:D],
                                              tp[:, h * D:(h + 1) * D])

            def emit_attn():
                q_sb, k_sb = proj["q"], proj["k"]
                for qh in range(2):
                    for h in range(HC):
                        hsl = slice(h * D, (h + 1) * D)
                        o_ps = [opool.tile([P, 512], FP32, tag="o",
                                           name=f"o_ps_h{h}_{qh}_{qb}")
                                for qb in range(2)]
                        for kt in range(NKT):
                            st = stpool.tile([P, 1024], FP32, tag="st")
                            for nb in range(2):
                                nc.tensor.matmul(
                                    st[:, ts(nb, 512)],
                                    k_sb[hsl, ts(kt, P)],
                                    q_sb[hsl, ds(qh * 1024 + nb * 512,
                                                   512)],
                                    start=True, stop=True)
                            e = epool.tile([P, 1024], FPR, tag="e")
                            nc.scalar.activation(e[:], st[:], AFT.Exp)
                            for qb in range(2):
                                nc.tensor.matmul(
                                    o_ps[qb][0:D + 1, :],
                                    vTa[:, h, kt, :],
                                    e[:, ts(qb, 512)],
                                    start=(kt == 0), stop=(kt == NKT - 1))
                        # normalize: rows 0:64 attn, row 64 denominator
                        for qb in range(2):
                            qsl = ds(qh * 1024 + qb * 512, 512)
                            rec = spool.tile([1, 512], FP32, tag="rec",
                                             name="rec")
                            nc.vector.reciprocal(rec[:],
                                                 o_ps[qb][D:D + 1, :])
                            bc = spool.tile([D, 512], FP32, tag="bc",
                                            name="bc")
                            nc.gpsimd.partition_broadcast(bc[:], rec[:])
                            nc.vector.tensor_mul(attn_sb[hsl, qsl],
                                                 o_ps[qb][0:D, :], bc[:])
                    nc.sync.dma_start(ag_in[qh][:],
                                      attn_sb[:, ds(qh * 1024, 1024)])

            def emit_pre():
                emit_xdma()
                emit_qkv()
                emit_vta()
                emit_attn()

            def emit_post():
                for nb in range(NB):
                    o_ps = opool.tile([P, 512], FP32, tag="o", name="out_ps")
                    for kc in range(KC):
                        rt = rpool.tile([P, 512], BF16, tag="rhs", name="rt")
                        nc.sync.dma_start(
                            rt[:], ag_out[nb // 2][ds(kc * P, P),
                                                   ts((nb % 2), 512)])
                        nc.tensor.matmul(o_ps[:], w_sb["o"][:, kc, :],
                                         rt[:],
                                         start=(kc == 0), stop=(kc == KC - 1))
                    nc.vector.tensor_scalar_add(out_sb[:, ts(nb, 512)],
                                                o_ps[:], bo_sb[:])
                    nc.sync.dma_start(out_d.ap()[:, ts(nb, 512)],
                                      out_sb[:, ts(nb, 512)])

            if loop_r is None:
                emit_pre()
            elif part == "full":
                with tc.For_i(0, loop_r, 1):
                    emit_pre()
            elif part == "xdma":
                with tc.For_i(0, loop_r, 1):
                    emit_xdma()
                emit_qkv(); emit_vta(); emit_attn()
            elif part == "qkv":
                with tc.For_i(0, loop_r, 1):
                    emit_xdma()
                    emit_qkv()
                emit_vta(); emit_attn()
            elif part == "attn":
                emit_xdma(); emit_qkv(); emit_vta()
                with tc.For_i(0, loop_r, 1):
                    emit_attn()
            elif part == "post":
                emit_pre()
            else:
                raise ValueError(part)
            for i in range(2):
                nc.gpsimd.collective_compute(
                    "AllGather", mybir.AluOpType.bypass,
                    ins=[ag_in[i].opt()], outs=[ag_out[i].opt()],
                    replica_groups=[list(range(N_CORES))])
            if loop_r is None or part != "post" and part != "full":
                emit_post()
            else:
                with tc.For_i(0, loop_r, 1):
                    emit_post()
    nc.finalize()
    return nc


_NC = None


def _get_nc():
    global _NC
    if _NC is None:
        _NC = build()
    return _NC


def make_in_maps(hidden_states, Wq, Wk, Wv, Wo, bo):
    x = np.ascontiguousarray(
        np.asarray(hidden_states, np.float32).reshape(C, S))
    scale = np.float32(D ** -0.5)
    Wq = np.asarray(Wq, np.float32)
    Wk = np.asarray(Wk, np.float32)
    Wv = np.asarray(Wv, np.float32)
    Wo = np.asarray(Wo, np.float32)
    bo = np.asarray(bo, np.float32)
    in_maps = []
    for i in range(N_CORES):
        sl = slice(i * P, (i + 1) * P)
        in_maps.append({
            "x": x.astype(ml_dtypes.bfloat16),
            "wqT": np.ascontiguousarray((Wq[sl] * scale).T).astype(ml_dtypes.bfloat16),
            "wkT": np.ascontiguousarray(Wk[sl].T).astype(ml_dtypes.bfloat16),
            "wvT": np.ascontiguousarray(Wv[sl].T).astype(ml_dtypes.bfloat16),
            "woT": np.ascontiguousarray(Wo[sl].T).astype(ml_dtypes.bfloat16),
            "bo": np.ascontiguousarray(bo[sl].reshape(P, 1)),
            "ident": np.eye(P, dtype=np.float32),
            "ones": np.ones((P, 1), np.float32),
        })
    return in_maps


def kernel(hidden_states, Wq, Wk, Wv, Wo, bo):
    nc = _get_nc()
    in_maps = make_in_maps(hidden_states, Wq, Wk, Wv, Wo, bo)
    res = run_bass_kernel_spmd(nc, in_maps, core_ids=list(range(N_CORES)))
    out = np.concatenate([res.results[i]["out"] for i in range(N_CORES)],
                         axis=0)
    return out.reshape(1, C, 1, S)


# revision 21
# speedup vs baseline: 4.8841x; 1.1619x over previous
"""Distributed multi-head attention kernel for 8 TRN2 NeuronCores.

Reference problem (hardcoded):
    hidden_states [1, 1024, 1, 2048] f32, Wq/Wk/Wv [1024, 1024],
    Wo [1024, 1024], bo [1024].  16 heads x 64 dim, seq 2048.

Sharding: tensor-parallel over heads.  Core i computes heads (2i, 2i+1):
  - QKV projections for its 128 channels (scale 1/8 folded into Wq),
  - scores transposed S_T[k, q] per head (no max subtraction; scores ~ N(0,1)),
  - one exp per key chunk on ScalarE ([128, 1024] double-buffered PSUM),
  - PV with a ones-column appended to vT so the softmax denominator falls out
    of the same PSUM accumulation,
  - normalize (reciprocal + partition-broadcast + multiply),
  - AllGather of the 1 MB attn block,
  - row shard of the output projection (+bias) -> out rows 128i..128(i+1).
Host concatenates the 8 row shards.

build(loop_r=N) wraps the pre-collective body and the post-collective
projection in hardware For_i loops (N iterations each) for wall-clock
benchmarking; the collective itself runs once (not allowed in control flow).
"""

import contextlib

import numpy as np
import ml_dtypes

import concourse.bass as bass
import concourse.mybir as mybir
import concourse.tile as tile
from concourse import bacc
from concourse.bass import ts, ds
from concourse.bass_utils import run_bass_kernel_spmd

# Problem constants (hardcoded per harness contract)
S = 2048          # sequence length
C = 1024          # query dim == inner dim
P = 128           # partitions / per-core channel count
D = 64            # head dim
HC = 2            # heads per core
N_CORES = 8
KC = C // P       # 8 contraction chunks for the projections
NKT = S // P      # 16 key-position chunks
NB = S // 512     # 4 free-dim blocks of 512
FP32 = mybir.dt.float32
FPR = mybir.dt.float32r
BF16 = mybir.dt.bfloat16
AFT = mybir.ActivationFunctionType


def build(loop_r=None, part="full"):
    nc = bacc.Bacc("TRN2", target_bir_lowering=False, debug=False,
                   num_devices=N_CORES)
    x_d = nc.dram_tensor("x", [C, S], BF16, kind="ExternalInput")
    wq_d = nc.dram_tensor("wqT", [C, P], BF16, kind="ExternalInput")
    wk_d = nc.dram_tensor("wkT", [C, P], BF16, kind="ExternalInput")
    wv_d = nc.dram_tensor("wvT", [C, P], BF16, kind="ExternalInput")
    wo_d = nc.dram_tensor("woT", [C, P], BF16, kind="ExternalInput")
    bo_d = nc.dram_tensor("bo", [P, 1], FP32, kind="ExternalInput")
    id_d = nc.dram_tensor("ident", [P, P], FPR, kind="ExternalInput")
    ones_d = nc.dram_tensor("ones", [P, 1], FPR, kind="ExternalInput")
    out_d = nc.dram_tensor("out", [P, S], FP32, kind="ExternalOutput")

    with tile.TileContext(nc) as tc:
        with (
            tc.tile_pool(name="const", bufs=1) as cpool,
            tc.tile_pool(name="big", bufs=1) as big,
            tc.tile_pool(name="opsum", bufs=4, space="PSUM") as opool,
            tc.tile_pool(name="stpsum", bufs=2, space="PSUM") as stpool,
            tc.tile_pool(name="exp", bufs=4) as epool,
            tc.tile_pool(name="small", bufs=4) as spool,
            tc.tile_pool(name="rhs", bufs=4) as rpool,
            tc.tile_pool(name="dram", bufs=1, space="DRAM") as dpool,
        ):
            # ---- constants / weights (outside any bench loop) ----
            ident = cpool.tile([P, P], FPR)
            nc.sync.dma_start(ident[:], id_d.ap())
            ones_sb = cpool.tile([P, 1], FPR, tag="ones")
            nc.sync.dma_start(ones_sb[:], ones_d.ap())
            w_sb = {}
            for name, dram in (("q", wq_d), ("k", wk_d), ("v", wv_d),
                               ("o", wo_d)):
                t = cpool.tile([P, KC, P], BF16, tag=f"w{name}")
                nc.sync.dma_start(
                    t[:], dram.ap().rearrange("(kc p) m -> p kc m", p=P))
                w_sb[name] = t
            bo_sb = cpool.tile([P, 1], FP32, tag="bo")
            nc.sync.dma_start(bo_sb[:], bo_d.ap())
            # absorb the exp table load into the DMA lead-in
            warm = cpool.tile([P, 1], FP32, tag="warm")
            nc.scalar.activation(warm[:], bo_sb[:], AFT.Exp)

            x_sb = big.tile([P, KC, S], BF16, tag="x")
            proj = {}
            for name in ("k", "q", "v"):
                proj[name] = big.tile([P, S], FPR, tag=f"{name}sb",
                                      name=f"{name}sb")
            vTa = big.tile([P, HC, NKT, D + 1], FPR, tag="vTa")
            attn_sb = big.tile([P, S], BF16, tag="attn")
            out_sb = big.tile([P, S], FP32, tag="outsb")
            ag_in = [dpool.tile([P, S // 2], BF16, tag=f"agin{i}",
                                name=f"agin{i}") for i in range(2)]
            ag_out = [dpool.tile([C, S // 2], BF16, tag=f"agout{i}",
                                 addr_space="Shared", name=f"agout{i}")
                      for i in range(2)]

            def emit_xdma():
                # x into SBUF in (kc, nb) sub-chunks so QKV chases the DMAs
                x_view = x_d.ap().rearrange("(kc p) s -> kc p s", kc=KC)
                for nb in range(NB):
                    for kc in range(KC):
                        nc.sync.dma_start(x_sb[:, kc, ts(nb, 512)],
                                          x_view[kc][:, ts(nb, 512)])

            def emit_qkv():
                # QKV projections, nb-major
                for nb in range(NB):
                    for name in ("k", "q", "v"):
                        ps = opool.tile([P, 512], FP32, tag="o", name="qkv_ps")
                        for kc in range(KC):
                            nc.tensor.matmul(
                                ps[:], w_sb[name][:, kc, :],
                                x_sb[:, kc, ts(nb, 512)],
                                start=(kc == 0), stop=(kc == KC - 1))
                        nc.vector.tensor_copy(proj[name][:, ts(nb, 512)],
                                              ps[:])
            def emit_vta():
                v_sb = proj["v"]
                # vT with ones column
                nc.vector.tensor_copy(
                    vTa[:, :, :, D:D + 1],
                    ones_sb[:, None, None, :].broadcast_to([P, HC, NKT, 1]))
                for kt in range(NKT):
                    tp = opool.tile([P, P], FPR, tag="o", name="tp")
                    nc.tensor.transpose(tp[:], v_sb[:, ts(kt, P)], ident[:])
                    for h in range(HC):
                        nc.vector.tensor_copy(vTa[:, h, kt, 0:D],
                                              tp[:, h * D:(h + 1) * D])

            def emit_attn():
                q_sb, k_sb = proj["q"], proj["k"]
                for qh in range(2):
                    for h in range(HC):
                        hsl = slice(h * D, (h + 1) * D)
                        o_ps = [opool.tile([P, 512], FP32, tag="o",
                                           name=f"o_ps_h{h}_{qh}_{qb}")
                                for qb in range(2)]
                        for kt in range(NKT):
                            st = stpool.tile([P, 1024], FP32, tag="st")
                            for nb in range(2):
                                nc.tensor.matmul(
                                    st[:, ts(nb, 512)],
                                    k_sb[hsl, ts(kt, P)],
                                    q_sb[hsl, ds(qh * 1024 + nb * 512,
                                                   512)],
                                    start=True, stop=True)
                            e = epool.tile([P, 1024], FPR, tag="e")
                            nc.scalar.activation(e[:], st[:], AFT.Exp)
                            for qb in range(2):
                                nc.tensor.matmul(
                                    o_ps[qb][0:D + 1, :],
                                    vTa[:, h, kt, :],
                                    e[:, ts(qb, 512)],
                                    start=(kt == 0), stop=(kt == NKT - 1))
                        # normalize: rows 0:64 attn, row 64 denominator
                        for qb in range(2):
                            qsl = ds(qh * 1024 + qb * 512, 512)
                            rec = spool.tile([1, 512], FP32, tag="rec",
                                             name="rec")
                            nc.vector.reciprocal(rec[:],
                                                 o_ps[qb][D:D + 1, :])
                            bc = spool.tile([D, 512], FP32, tag="bc",
                                            name="bc")
                            nc.gpsimd.partition_broadcast(bc[:], rec[:])
                            nc.vector.tensor_mul(attn_sb[hsl, qsl],
                                                 o_ps[qb][0:D, :], bc[:])
                    nc.sync.dma_start(ag_in[qh][:],
                                      attn_sb[:, ds(qh * 1024, 1024)])

            def emit_pre():
                emit_xdma()
                emit_qkv()
                emit_vta()
                emit_attn()

            def emit_post():
                for nb in range(2):
                    o_ps = [opool.tile([P, 512], FP32, tag="o",
                                       name=f"out_ps{nb}_{j}")
                            for j in range(2)]
                    for kc in range(KC):
                        rt = rpool.tile([P, 1024], BF16, tag="rhs", name="rt")
                        nc.sync.dma_start(
                            rt[:], ag_out[nb][ds(kc * P, P), :])
                        for j in range(2):
                            nc.tensor.matmul(
                                o_ps[j][:], w_sb["o"][:, kc, :],
                                rt[:, ts(j, 512)],
                                start=(kc == 0), stop=(kc == KC - 1))
                    for j in range(2):
                        nc.vector.tensor_scalar_add(
                            out_sb[:, ds(nb * 1024 + j * 512, 512)],
                            o_ps[j][:], bo_sb[:])
                    nc.sync.dma_start(out_d.ap()[:, ts(nb, 1024)],
                                      out_sb[:, ts(nb, 1024)])

            if loop_r is None:
                emit_pre()
            elif part == "full":
                with tc.For_i(0, loop_r, 1):
                    emit_pre()
            elif part == "xdma":
                with tc.For_i(0, loop_r, 1):
                    emit_xdma()
                emit_qkv(); emit_vta(); emit_attn()
            elif part == "qkv":
                with tc.For_i(0, loop_r, 1):
                    emit_xdma()
                    emit_qkv()
                emit_vta(); emit_attn()
            elif part == "attn":
                emit_xdma(); emit_qkv(); emit_vta()
                with tc.For_i(0, loop_r, 1):
                    emit_attn()
            elif part == "post":
                emit_pre()
            else:
                raise ValueError(part)
            for i in range(2):
                nc.gpsimd.collective_compute(
                    "AllGather", mybir.AluOpType.bypass,
                    ins=[ag_in[i].opt()], outs=[ag_out[i].opt()],
                    replica_groups=[list(range(N_CORES))])
            if loop_r is None or part != "post" and part != "full":
                emit_post()
            else:
                with tc.For_i(0, loop_r, 1):
                    emit_post()
    nc.finalize()
    return nc


_NC = None


def _get_nc():
    global _NC
    if _NC is None:
        _NC = build()
    return _NC


def make_in_maps(hidden_states, Wq, Wk, Wv, Wo, bo):
    x = np.ascontiguousarray(
        np.asarray(hidden_states, np.float32).reshape(C, S))
    scale = np.float32(D ** -0.5)
    Wq = np.asarray(Wq, np.float32)
    Wk = np.asarray(Wk, np.float32)
    Wv = np.asarray(Wv, np.float32)
    Wo = np.asarray(Wo, np.float32)
    bo = np.asarray(bo, np.float32)
    in_maps = []
    for i in range(N_CORES):
        sl = slice(i * P, (i + 1) * P)
        in_maps.append({
            "x": x.astype(ml_dtypes.bfloat16),
            "wqT": np.ascontiguousarray((Wq[sl] * scale).T).astype(ml_dtypes.bfloat16),
            "wkT": np.ascontiguousarray(Wk[sl].T).astype(ml_dtypes.bfloat16),
            "wvT": np.ascontiguousarray(Wv[sl].T).astype(ml_dtypes.bfloat16),
            "woT": np.ascontiguousarray(Wo[sl].T).astype(ml_dtypes.bfloat16),
            "bo": np.ascontiguousarray(bo[sl].reshape(P, 1)),
            "ident": np.eye(P, dtype=np.float32),
            "ones": np.ones((P, 1), np.float32),
        })
    return in_maps


def kernel(hidden_states, Wq, Wk, Wv, Wo, bo):
    nc = _get_nc()
    in_maps = make_in_maps(hidden_states, Wq, Wk, Wv, Wo, bo)
    res = run_bass_kernel_spmd(nc, in_maps, core_ids=list(range(N_CORES)))
    out = np.concatenate([res.results[i]["out"] for i in range(N_CORES)],
                         axis=0)
    return out.reshape(1, C, 1, S)
